# revision 57
# baseline (speedup 1.0000x reference)
"""Trainium2 Bass kernel for a dense transformer encoder layer.

Problem (hardcoded): x [2, 2048, 1024], 16 heads, FFN 4096, fp32,
post-LN residual blocks, mask additively applied before softmax.

Sharding: sequence-parallel over the 4096 tokens -> 512 tokens per core
(cores 0-3 handle batch 0, cores 4-7 batch 1). Every core computes the
full-batch K/V projections itself, keeps K^T / V' entirely in SBUF
(bf16), then runs attention for its own 512 queries, the output
projection (folded into the attention head loop via an SBUF
accumulator), LN1, the FFN and LN2.

All matmul operands are bf16 (same PE rate as fp32r at free-dim 512 but
half the DMA/SBUF traffic and fast weight loads); accumulation is fp32
in PSUM. Softmax/LN arithmetic stays fp32.

Matmul layouts (out = lhsT.T @ rhs, contraction on the partition dim):
  Q^T/K^T : lhsT = W k/m-tile [din,dout], rhs = x^T [din,tok]  -> [dout,tok]
  V       : lhsT = x^T [din,tok],  rhs = Wv [din,dout]         -> [tok,dout]
  scoresT : lhsT = K^T head [dh,kpos], rhs = Q^T head [dh,qpos]-> [kpos,qpos]
            (head pairs run concurrently in disjoint PE row groups)
  attn@V' : lhsT = V' [kpos,dh+1], rhs = expT [kpos,qpos]      -> [dh+1,qpos]
            (V' has a ones column -> row dh is the softmax denominator)
  outproj : lhsT = o^T [din,q], rhs = Wp [din,dout]            -> [q,dout]
  FFN1    : lhsT = W1 [din,dffn], rhs = xln1^T [din,q]         -> [dffn,q]
  FFN2    : lhsT = h^T [dffn,q], rhs = W2 [dffn,dout]          -> [q,dout]

Host-side exact folds: bp' = bp + bv @ Wp into the residual (attention
with V-bias == attention without + bv once rows sum to 1), so bv/bp
never touch the device.
"""

from contextlib import ExitStack

import numpy as np

import concourse.bass as bass
import concourse.mybir as mybir
import concourse.tile as tile
from concourse.bass_utils import run_bass_kernel_spmd
from concourse.masks import make_identity
from concourse.vector_clock import ScopedClock

FP32 = mybir.dt.float32
FP32R = mybir.dt.float32r
BF16 = mybir.dt.bfloat16
F8E4 = mybir.dt.float8e4
DR = mybir.MatmulPerfMode.DoubleRow
AF = mybir.ActivationFunctionType
ALU = mybir.AluOpType
WS = 16.0          # fp8 weight pre-scale (host), compensated in drains

P = 128
D = 1024
F = 4096
H = 16
DH = 64
S = 2048          # tokens per batch
TPC = 512         # tokens (queries) per core
NB = D // P       # 8 dout blocks
KB = D // P       # 8 contraction tiles over D
FB = F // P       # 32 dffn tiles
QT = TPC // P     # 4 query tiles
KT16 = S // P     # 16 kpos tiles
NG = S // TPC     # 4 kpos 512-slices
VW = H * (DH + 1)  # 1040: V' row width
SCALE = DH ** -0.5
EPS = 1e-6
N_CORES = 8


# --- Tile tail-drain fix: this walrus build allows only one sem-wait per
# instruction; Tile's final drain accumulates several. Split them across
# dedicated nops before draining.
def _patched_drain_and_barrier(self, tick_clock, wait_clock):
    probe = self.nc.sync.nop(nofuse=True, hint="drain_wait_split")
    wait_clock.add_sem_waits(probe.ins, ScopedClock({None: tick_clock.global_clock}))
    si = probe.ins.sync_info
    if si is not None and si.on_wait and len(si.on_wait) > 1:
        waits = list(si.on_wait)
        si.on_wait = waits[:1]
        for w in waits[1:]:
            extra = self.nc.sync.nop(nofuse=True, hint="drain_wait_split")
            esi = extra.ins.sync_info
            if esi is None:
                extra.ins.sync_info = mybir.SyncInfo(on_wait=[w], on_update=[])
            else:
                esi.on_wait = [w]
    self.nc.sync.drain()
    self.nc.all_engine_barrier()
    assert self.sems is not None
    popped = self.nc._tile_sem_poison_stack.pop()
    assert popped is self._sem_poison
    self.nc.clear_and_free_semaphores(list(self.sems.allocated().values()))
    self.nc.all_engine_barrier()


if getattr(tile.TileContext, "_drain_patch", None) is None:
    tile.TileContext._drain_and_barrier = _patched_drain_and_barrier
    tile.TileContext._drain_patch = True


def _r(ap):
    return ap.bitcast(FP32R)


def _split_waits(nc):
    """Walrus codegen accepts at most one sem-wait per instruction (two on
    EventSemaphore). Tile's scheduler can emit more; hoist the surplus onto
    same-engine EventSemaphore instructions inserted just before."""
    uid = [0]
    for bb in nc.m.functions[0].blocks:
        new_insts = []
        for inst in bb.instructions:
            si = inst.sync_info
            limit = 2 if isinstance(inst, mybir.InstEventSemaphore) else 1
            if si is not None and si.on_wait and len(si.on_wait) > limit:
                waits = list(si.on_wait)
                extra, keep = waits[:-limit], waits[-limit:]
                for i in range(0, len(extra), 2):
                    uid[0] += 1
                    ev = mybir.InstEventSemaphore(
                        name=f"I-wsplit-{uid[0]}",
                        engine=inst.engine,
                        sync_info=mybir.SyncInfo(
                            on_wait=extra[i:i + 2], on_update=[]),
                    )
                    nc.register_instruction(ev)
                    new_insts.append(ev)
                si.on_wait = keep
            new_insts.append(inst)
        if len(new_insts) != len(bb.instructions):
            bb.instructions[:] = new_insts


def _ln_chain(nc, pool, y, s1, out_ap, gamma_b, beta_b):
    """LayerNorm over the free dim of y [128, D] (torch semantics: unbiased
    std, denominator std + eps), given s1 = row-sums of y. Uses
    var = (E[y^2]*D - D*mean^2)/(D-1) so the Square pass runs concurrently
    with the mean computation. Writes out_ap (any dtype)."""
    sq = pool.tile([P, D], FP32, tag="ln_sq")
    ss2 = pool.tile([P, 1], FP32, tag="ln_ss2")
    nc.scalar.activation(sq[:], y[:], AF.Square, accum_out=ss2[:])
    mn = pool.tile([P, 1], FP32, tag="ln_mn")
    nc.scalar.mul(mn[:], s1[:], 1.0 / D)
    mn2 = pool.tile([P, 1], FP32, tag="ln_mn2")
    nc.vector.tensor_scalar_mul(mn2[:], mn[:], mn[:])
    var = pool.tile([P, 1], FP32, tag="ln_var")
    nc.vector.scalar_tensor_tensor(
        var[:], mn2[:], -float(D), ss2[:], op0=ALU.mult, op1=ALU.add)
    std = pool.tile([P, 1], FP32, tag="ln_std")
    nc.scalar.activation(std[:], var[:], AF.Sqrt, scale=1.0 / (D - 1))
    nc.vector.tensor_scalar_add(std[:], std[:], EPS)
    rcp = pool.tile([P, 1], FP32, tag="ln_rcp")
    nc.vector.reciprocal(rcp[:], std[:])
    t1 = pool.tile([P, D], FP32, tag="ln_t1")
    nc.vector.scalar_tensor_tensor(
        t1[:], y[:], mn[:], gamma_b[:], op0=ALU.subtract, op1=ALU.mult)
    nc.vector.scalar_tensor_tensor(
        out_ap, t1[:], rcp[:], beta_b[:], op0=ALU.mult, op1=ALU.add)


def build_program(use_mask: bool) -> bass.Bass:
    nc = bass.Bass(target_bir_lowering=False, debug=False)

    # ---- I/O ----
    xT_d = nc.dram_tensor("xT", [D, S], F8E4, kind="ExternalInput")
    xTq_d = nc.dram_tensor("xTq", [D, TPC], F8E4, kind="ExternalInput")
    xres_d = nc.dram_tensor("xres", [TPC, D], FP32, kind="ExternalInput")
    wq_d = nc.dram_tensor("wq", [D, D], F8E4, kind="ExternalInput")
    wk_d = nc.dram_tensor("wk", [D, D], F8E4, kind="ExternalInput")
    wv_d = nc.dram_tensor("wv", [D, D], F8E4, kind="ExternalInput")
    wp_d = nc.dram_tensor("wp", [D, D], F8E4, kind="ExternalInput")
    w1_d = nc.dram_tensor("w1", [D, F], BF16, kind="ExternalInput")
    w2_d = nc.dram_tensor("w2", [F, D], BF16, kind="ExternalInput")
    bq_d = nc.dram_tensor("bq", [D], FP32, kind="ExternalInput")
    bk_d = nc.dram_tensor("bk", [D], FP32, kind="ExternalInput")
    b1_d = nc.dram_tensor("b1", [F], FP32, kind="ExternalInput")
    b2_d = nc.dram_tensor("b2", [D], FP32, kind="ExternalInput")
    g1_d = nc.dram_tensor("g1", [D], FP32, kind="ExternalInput")
    be1_d = nc.dram_tensor("be1", [D], FP32, kind="ExternalInput")
    g2_d = nc.dram_tensor("g2", [D], FP32, kind="ExternalInput")
    be2_d = nc.dram_tensor("be2", [D], FP32, kind="ExternalInput")
    if use_mask:
        maskT_d = nc.dram_tensor("maskT", [S, TPC], FP32, kind="ExternalInput")
    out_d = nc.dram_tensor("out", [TPC, D], FP32, kind="ExternalOutput")

    with tile.TileContext(nc) as tc:
        _build_body(
            nc, tc, use_mask,
            xT_d, xTq_d, xres_d, wq_d, wk_d, wv_d, wp_d, w1_d, w2_d,
            bq_d, bk_d, b1_d, b2_d, g1_d, be1_d, g2_d, be2_d,
            maskT_d if use_mask else None, out_d,
        )
    _split_waits(nc)
    return nc


def _build_body(nc, tc, use_mask, xT_d, xTq_d, xres_d, wq_d, wk_d, wv_d,
                wp_d, w1_d, w2_d, bq_d, bk_d, b1_d, b2_d, g1_d, be1_d,
                g2_d, be2_d, maskT_d, out_d):
    with ExitStack() as top:
        consts = top.enter_context(tc.tile_pool(name="consts", bufs=1))
        ident = consts.tile([P, P], BF16)
        ones_row = consts.tile([1, DH], BF16)
        nc.vector.memset(ones_row[:], 1.0)
        ones_col = consts.tile([1, P], BF16)
        nc.vector.memset(ones_col[:], 1.0)
        bq_c = consts.tile([P, NB], FP32)
        nc.sync.dma_start(bq_c[:], bq_d.ap().rearrange("(b p) -> p b", p=P))
        bk_c = consts.tile([P, NB], FP32)
        nc.sync.dma_start(bk_c[:], bk_d.ap().rearrange("(b p) -> p b", p=P))
        b1_c = consts.tile([P, FB], FP32)
        nc.sync.dma_start(b1_c[:], b1_d.ap().rearrange("(b p) -> p b", p=P))


        persist = top.enter_context(tc.tile_pool(name="persist", bufs=1))
        ot_sb = persist.tile([P, NB * TPC], F8E4)    # o^T normalized (x16), 4KB
        wp_sb = persist.tile([P, KB * D], F8E4)      # Wp (x16), 8KB
        for b in range(KB):
            nc.sync.dma_start(wp_sb[:, b * D:(b + 1) * D],
                              wp_d.ap()[b * P:(b + 1) * P, :])

        # ================= QKV + attention =================
        with ExitStack() as mid:
            attn_sb = mid.enter_context(tc.tile_pool(name="attn_sb", bufs=1))
            qt_sb = attn_sb.tile([P, NB * TPC], BF16)   # Q^T (+bq), 8KB/part
            kt_sb = attn_sb.tile([P, NB * S], BF16)     # K^T (+bk), 32KB/part
            vp_sb = attn_sb.tile([P, KT16 * VW], F8E4)  # V' (x16), 16.25KB

            # V' ones columns (= WS: V' carries the x16 weight scale, so the
            # denominator row is 16*d and the numerators 16*(v.et) -- the
            # scale cancels in the softmax divide)
            for kt in range(KT16):
                v3 = vp_sb[:, kt * VW:(kt + 1) * VW].rearrange(
                    "p (h j) -> p h j", j=DH + 1)
                nc.vector.memset(v3[:, :, DH:DH + 1], WS)

            xp = mid.enter_context(tc.tile_pool(name="xt", bufs=1))
            wpool = mid.enter_context(tc.tile_pool(name="qkv_w", bufs=2))
            qkv_ps = mid.enter_context(
                tc.tile_pool(name="qkv_ps", bufs=2, space="PSUM"))

            # --- Q^T -> qt_sb (+bq) --- own 512 query columns only
            # (xtq/wq DMAs issued first so the first matmuls start ASAP)
            xtq = xp.tile([P, KB * TPC], F8E4)
            for b in range(KB):
                nc.sync.dma_start(xtq[:, b * TPC:(b + 1) * TPC],
                                  xTq_d.ap()[b * P:(b + 1) * P, :])
            w_sb = wpool.tile([P, KB * D], F8E4, tag="w")
            for b in range(KB):
                nc.gpsimd.dma_start(w_sb[:, b * D:(b + 1) * D],
                                    wq_d.ap()[b * P:(b + 1) * P, :])
            xt = xp.tile([P, KB * S], F8E4)          # x^T full batch, 16KB
            for b in range(KB):
                nc.sync.dma_start(xt[:, b * S:(b + 1) * S],
                                  xT_d.ap()[b * P:(b + 1) * P, :])
            xt3 = xt[:].rearrange("p (k s) -> p k s", s=S)
            xtq3 = xtq[:].rearrange("p (k t) -> p k t", t=TPC)
            w3 = w_sb[:].rearrange("p (k d) -> p k d", d=D)
            for m in range(NB):
                ps = qkv_ps.tile([P, TPC], FP32, tag="qkvps")
                for k in range(0, KB, 2):
                    nc.tensor.matmul(
                        ps[:],
                        lhsT=w3[:, k:k + 2, m * P:(m + 1) * P],
                        rhs=xtq3[:, k:k + 2, :],
                        start=(k == 0), stop=(k == KB - 2), perf_mode=DR,
                    )
                nc.vector.tensor_scalar(
                    qt_sb[:, m * TPC:(m + 1) * TPC], ps[:], 1.0 / WS,
                    bq_c[:, m:m + 1], op0=ALU.mult, op1=ALU.add)

            # --- K^T -> kt_sb (+bk), head-pair-major; pair 0 computed
            # before V so attention head-pair 0 can start immediately ---
            wk_sb = wpool.tile([P, KB * D], F8E4, tag="w")
            for b in range(KB):
                nc.gpsimd.dma_start(wk_sb[:, b * D:(b + 1) * D],
                                    wk_d.ap()[b * P:(b + 1) * P, :])
            wk3 = wk_sb[:].rearrange("p (k d) -> p k d", d=D)

            def k_block(m):
                for ng in range(NG):
                    ps = qkv_ps.tile([P, TPC], FP32, tag="qkvps",
                                     name=f"kps{m}{ng}")
                    for k in range(0, KB, 2):
                        nc.tensor.matmul(
                            ps[:],
                            lhsT=wk3[:, k:k + 2, m * P:(m + 1) * P],
                            rhs=xt3[:, k:k + 2, ng * TPC:(ng + 1) * TPC],
                            start=(k == 0), stop=(k == KB - 2), perf_mode=DR,
                        )
                    nc.vector.tensor_scalar(
                        kt_sb[:, m * S + ng * TPC: m * S + (ng + 1) * TPC],
                        ps[:], 1.0 / WS, bk_c[:, m:m + 1],
                        op0=ALU.mult, op1=ALU.add)

            k_block(0)
            k_block(1)

            # --- V -> vp_sb (no bias; bv folded into xres on host), with
            # the remaining K head-pair blocks interleaved so K stays ahead
            # of the attention wave ---
            w_sb = wpool.tile([P, KB * D], F8E4, tag="w")
            for b in range(KB):
                nc.gpsimd.dma_start(w_sb[:, b * D:(b + 1) * D],
                                    wv_d.ap()[b * P:(b + 1) * P, :])
            w3 = w_sb[:].rearrange("p (k d) -> p k d", d=D)
            next_k = 2
            for mt in range(KT16):
                if mt % 3 == 2 and next_k < NB:
                    k_block(next_k)
                    next_k += 1
                v3 = vp_sb[:, mt * VW:(mt + 1) * VW].rearrange(
                    "p (h j) -> p h j", j=DH + 1)
                for nd in range(2):
                    ps = qkv_ps.tile([P, TPC], FP32, tag="qkvps")
                    for k in range(0, KB, 2):
                        nc.tensor.matmul(
                            ps[:],
                            lhsT=xt3[:, k:k + 2, mt * P:(mt + 1) * P],
                            rhs=w3[:, k:k + 2, nd * TPC:(nd + 1) * TPC],
                            start=(k == 0), stop=(k == KB - 2), perf_mode=DR,
                        )
                    # V' keeps the x16 weight scale (cancels in softmax)
                    nc.vector.tensor_copy(
                        v3[:, nd * 8:(nd + 1) * 8, 0:DH],
                        ps[:].rearrange("p (h j) -> p h j", j=DH))
            while next_k < NB:
                k_block(next_k)
                next_k += 1

            # --- attention ---
            sp2p = mid.enter_context(
                tc.tile_pool(name="sp2", bufs=2, space="PSUM"))
            opp = mid.enter_context(
                tc.tile_pool(name="opps", bufs=2, space="PSUM"))
            etp = mid.enter_context(tc.tile_pool(name="et", bufs=6))
            scr = mid.enter_context(tc.tile_pool(name="attn_scr", bufs=2))
            if use_mask:
                mkp = mid.enter_context(tc.tile_pool(name="mk", bufs=3))

            def normalize(hb, hpar, op):
                """softmax divide for head (2*hb + hpar) -> ot_sb (bf16)."""
                hp = hpar * DH
                rr = scr.tile([1, TPC], FP32, tag="rr")
                nc.vector.reciprocal(rr[:], op[DH:DH + 1, :])
                rrb = scr.tile([1, TPC], BF16, tag="rrb")
                # x WS so ot lands in fp8's normal range (undone in proj drain)
                nc.vector.tensor_scalar(rrb[:], rr[:], WS, None, op0=ALU.mult)
                rb_ps = qkv_ps.tile([DH, TPC], FP32, tag="qkvps")
                nc.tensor.matmul(rb_ps[:], lhsT=ones_row[:],
                                 rhs=rrb[:], start=True, stop=True)
                rb_sb = scr.tile([DH, TPC], BF16, tag="rbsb")
                nc.vector.tensor_copy(rb_sb[:], rb_ps[:])
                nc.vector.tensor_mul(
                    ot_sb[hp:hp + DH, hb * TPC:(hb + 1) * TPC],
                    op[0:DH, :], rb_sb[:])

            vp3 = vp_sb[:].rearrange("p (kt w) -> p kt w", w=VW)
            pending = None  # (hb, [op0, op1]) awaiting normalize
            attnv_q = []   # deferred attnV pairs: (hb, ktp, ops, et4r)
            NKP = KT16 // 2

            def emit_attnv(hb, ktp, ops, et4r):
                for hpar in range(2):
                    h = 2 * hb + hpar
                    nc.tensor.matmul(
                        ops[hpar][:],
                        lhsT=vp3[:, 2 * ktp:2 * ktp + 2,
                                 h * (DH + 1):(h + 1) * (DH + 1)],
                        rhs=et4r[:, :, hpar, :],
                        start=(ktp == 0), stop=(ktp == NKP - 1),
                        perf_mode=DR,
                    )

            for hb in range(NB):
                ops = [opp.tile([DH + 1, TPC], FP32, tag="op",
                                name=f"op_{hb}_{i}") for i in range(2)]
                for ktp in range(NKP):
                    # exp tiles for a kt pair: layout [p, kt-sub, head, q]
                    et4 = etp.tile([P, 4 * TPC], F8E4, tag="et")
                    for sub in range(2):
                        kt = 2 * ktp + sub
                        sp = sp2p.tile([P, 2 * TPC], FP32, tag="sp")
                        for hpar in range(2):
                            hp = hpar * DH
                            nc.tensor.matmul(
                                sp[:, hpar * TPC:(hpar + 1) * TPC],
                                lhsT=kt_sb[hp:hp + DH, hb * S + kt * P:
                                           hb * S + (kt + 1) * P],
                                rhs=qt_sb[hp:hp + DH, hb * TPC:(hb + 1) * TPC],
                                start=True, stop=True,
                            )
                        if use_mask:
                            mk = mkp.tile([P, TPC], FP32, tag="mk")
                            nc.sync.dma_start(
                                mk[:], maskT_d.ap()[kt * P:(kt + 1) * P, :])
                            for hpar in range(2):
                                nc.vector.tensor_add(
                                    sp[:, hpar * TPC:(hpar + 1) * TPC],
                                    sp[:, hpar * TPC:(hpar + 1) * TPC], mk[:])
                        nc.scalar.activation(
                            et4[:, sub * 2 * TPC:(sub + 1) * 2 * TPC], sp[:],
                            AF.Exp, scale=SCALE)
                        # previous head pair's softmax divide, emitted behind
                        # this pair's first scores/exp so it never
                        # head-of-line-blocks the PE queue at the boundary
                        if pending is not None and ktp == 0:
                            normalize(pending[0], sub, pending[1][sub])
                            if sub == 1:
                                pending = None
                    et4r = et4[:].rearrange("p (kp h t) -> p kp h t",
                                            kp=2, h=2)
                    # software pipeline: attnV runs one kt-pair behind the
                    # scores/exp stream so the PE never delays the next exp
                    attnv_q.append((hb, ktp, ops, et4r))
                    if len(attnv_q) > 1:
                        emit_attnv(*attnv_q.pop(0))
                # flush before the next head pair: the normalize emitted
                # inside its kt loop must come after ALL op accumulations
                while attnv_q:
                    emit_attnv(*attnv_q.pop(0))
                pending = (hb, ops)
            normalize(pending[0], 0, pending[1][0])
            normalize(pending[0], 1, pending[1][1])

        # ================= LN1 + transpose + FFN + LN2 =================
        with ExitStack() as tail:
            big = tail.enter_context(tc.tile_pool(name="tail_big", bufs=1))
            xres_sb = big.tile([P, QT * D], FP32)    # residual + bp + bv@Wp
            for qt in range(QT):
                nc.sync.dma_start(xres_sb[:, qt * D:(qt + 1) * D],
                                  xres_d.ap()[qt * P:(qt + 1) * P, :])
            xln1 = big.tile([P, QT * D], BF16)       # LN1 output (natural)
            xln1T = big.tile([P, KB * TPC], BF16)    # its transpose
            hT = big.tile([P, FB * TPC], BF16)       # relu(x@W1+b1)^T, 32KB
            b2row = big.tile([1, D], BF16)
            nc.gpsimd.dma_start(b2row[:], b2_d.ap()[None, :])
            g1_b = big.tile([P, D], FP32)
            nc.sync.dma_start(g1_b[:], g1_d.ap()[None, :].to_broadcast((P, D)))
            be1_b = big.tile([P, D], FP32)
            nc.sync.dma_start(be1_b[:], be1_d.ap()[None, :].to_broadcast((P, D)))
            g2_b = big.tile([P, D], FP32)
            nc.sync.dma_start(g2_b[:], g2_d.ap()[None, :].to_broadcast((P, D)))
            be2_b = big.tile([P, D], FP32)
            nc.sync.dma_start(be2_b[:], be2_d.ap()[None, :].to_broadcast((P, D)))

            # --- output projection: o @ Wp accumulated across hb in PSUM,
            # drained directly into the LN1 y-assembly ---
            lnp = tail.enter_context(tc.tile_pool(name="ln_scr", bufs=2))
            yp = tail.enter_context(tc.tile_pool(name="ln_y", bufs=1))
            ys = []
            with tc.tile_pool(name="pj_ps", bufs=8, space="PSUM") as pjp:
                pj = {(qt, nd): pjp.tile([P, TPC], FP32, tag="pj",
                                         name=f"pj_{qt}_{nd}")
                      for qt in range(QT) for nd in range(2)}
                ot3 = ot_sb[:].rearrange("p (b t) -> p b t", t=TPC)
                wp3 = wp_sb[:].rearrange("p (b d) -> p b d", d=D)
                for hb in range(0, NB, 2):
                    for qt in range(QT):
                        for nd in range(2):
                            nc.tensor.matmul(
                                pj[qt, nd][:],
                                lhsT=ot3[:, hb:hb + 2, qt * P:(qt + 1) * P],
                                rhs=wp3[:, hb:hb + 2, nd * TPC:(nd + 1) * TPC],
                                start=(hb == 0), stop=(hb == NB - 2),
                                perf_mode=DR,
                            )
                for qt in range(QT):
                    y = yp.tile([P, D], FP32, tag=f"y{qt}", name=f"y{qt}")
                    sh = [yp.tile([P, 1], FP32, tag=f"sh{qt}{nd}",
                                  name=f"sh{qt}{nd}")
                          for nd in range(2)]
                    for nd in range(2):
                        nc.vector.scalar_tensor_tensor(
                            y[:, nd * TPC:(nd + 1) * TPC], pj[qt, nd][:],
                            1.0 / (WS * WS),
                            xres_sb[:, qt * D + nd * TPC:
                                    qt * D + (nd + 1) * TPC],
                            op0=ALU.mult, op1=ALU.add, accum_out=sh[nd][:])
                    s1 = yp.tile([P, 1], FP32, tag=f"s1{qt}", name=f"s1{qt}")
                    nc.vector.tensor_add(s1[:], sh[0][:], sh[1][:])
                    ys.append((y, s1))

            make_identity(nc, ident[:])
            with tc.tile_pool(name="tp_ps", bufs=2, space="PSUM") as tpp:
                for qt in range(QT):
                    y, s1 = ys[qt]
                    _ln_chain(nc, lnp, y, s1,
                              xln1[:, qt * D:(qt + 1) * D], g1_b, be1_b)
                    for bd in range(NB):
                        tp = tpp.tile([P, P], BF16, tag="tps")
                        nc.tensor.transpose(
                            tp[:],
                            xln1[:, qt * D + bd * P: qt * D + (bd + 1) * P],
                            ident[:])
                        nc.vector.tensor_copy(
                            xln1T[:, bd * TPC + qt * P: bd * TPC + (qt + 1) * P],
                            tp[:])

            fwp = tail.enter_context(tc.tile_pool(name="ffn_w", bufs=4))
            with tc.tile_pool(name="ffn1_ps", bufs=4, space="PSUM") as fps:
                for mf in range(FB):
                    w1t = fwp.tile([P, KB * P], BF16, tag="w1t")
                    nc.sync.dma_start(
                        w1t[:].rearrange("p (k c) -> p k c", c=P),
                        w1_d.ap()[:, mf * P:(mf + 1) * P].rearrange(
                            "(k p) c -> p k c", p=P))
                    ph = fps.tile([P, TPC], FP32, tag="fps")
                    for k in range(KB):
                        nc.tensor.matmul(
                            ph[:],
                            lhsT=w1t[:, k * P:(k + 1) * P],
                            rhs=xln1T[:, k * TPC:(k + 1) * TPC],
                            start=(k == 0), stop=(k == KB - 1),
                        )
                    nc.scalar.activation(
                        hT[:, mf * TPC:(mf + 1) * TPC], ph[:], AF.Relu,
                        bias=b1_c[:, mf:mf + 1])

            # FFN2 split into two qt-halves so LN2 of the first half
            # overlaps the second half's matmuls (w2 is streamed twice).
            with tc.tile_pool(name="ffn2_ps", bufs=4, space="PSUM") as fp2:
                for half in range(2):
                    qts = (0, 1) if half == 0 else (2, 3)
                    pj2 = {(qt, nd): fp2.tile([P, TPC], FP32, tag="f2ps",
                                              name=f"pj2_{qt}_{nd}")
                           for qt in qts for nd in range(2)}
                    for k2 in range(FB):
                        w2t = fwp.tile([P, D], BF16, tag="w2t")
                        nc.sync.dma_start(w2t[:],
                                          w2_d.ap()[k2 * P:(k2 + 1) * P, :])
                        for qt in qts:
                            for nd in range(2):
                                nc.tensor.matmul(
                                    pj2[qt, nd][:],
                                    lhsT=hT[:, k2 * TPC + qt * P:
                                            k2 * TPC + (qt + 1) * P],
                                    rhs=w2t[:, nd * TPC:(nd + 1) * TPC],
                                    start=(k2 == 0), stop=False,
                                )
                    for qt in qts:
                        # + b2 via ones-row matmul (closes the accumulation)
                        for nd in range(2):
                            nc.tensor.matmul(
                                pj2[qt, nd][:], lhsT=ones_col[:],
                                rhs=b2row[0:1, nd * TPC:(nd + 1) * TPC],
                                start=False, stop=True,
                            )
                        y2 = lnp.tile([P, D], FP32, tag="ln_y")
                        sh = [lnp.tile([P, 1], FP32, tag=f"ln_sh{nd}",
                                       name=f"sh2_{qt}{nd}")
                              for nd in range(2)]
                        for nd in range(2):
                            nc.vector.scalar_tensor_tensor(
                                y2[:, nd * TPC:(nd + 1) * TPC], pj2[qt, nd][:],
                                0.0,
                                xln1[:, qt * D + nd * TPC:
                                     qt * D + (nd + 1) * TPC],
                                op0=ALU.add, op1=ALU.add, accum_out=sh[nd][:])
                        s1 = lnp.tile([P, 1], FP32, tag="ln_s1")
                        nc.vector.tensor_add(s1[:], sh[0][:], sh[1][:])
                        yo = lnp.tile([P, D], FP32, tag="ln_yo")
                        _ln_chain(nc, lnp, y2, s1, yo[:], g2_b, be2_b)
                        nc.sync.dma_start(out_d.ap()[qt * P:(qt + 1) * P, :],
                                          yo[:])


_PROG_CACHE: dict = {}


def _get_program(use_mask: bool) -> bass.Bass:
    if use_mask not in _PROG_CACHE:
        _PROG_CACHE[use_mask] = build_program(use_mask)
    return _PROG_CACHE[use_mask]


def make_in_maps(x, mask, Wq, bq, Wk, bk, Wv, bv, Wp, bp,
                 gamma1, beta1, W1, b1, W2, b2, gamma2, beta2):
    import ml_dtypes
    BF = ml_dtypes.bfloat16
    F8 = ml_dtypes.float8_e4m3
    ws = np.float32(16.0)  # must match kernel WS

    x = np.asarray(x, np.float32)
    mask = np.asarray(mask)
    use_mask = not bool(mask.all())
    Wp32 = np.ascontiguousarray(Wp, np.float32)
    # exact fold: attention(V + bv) == attention(V) + bv (softmax rows sum
    # to 1), so o@Wp + bp == o_nobias@Wp + (bv@Wp + bp); fold into residual.
    res_bias = (np.asarray(bv, np.float32) @ Wp32
                + np.asarray(bp, np.float32)).astype(np.float32)
    common = {
        "wq": (np.ascontiguousarray(Wq, np.float32) * ws).astype(F8),
        "wk": (np.ascontiguousarray(Wk, np.float32) * ws).astype(F8),
        "wv": (np.ascontiguousarray(Wv, np.float32) * ws).astype(F8),
        "wp": (Wp32 * ws).astype(F8),
        "w1": np.ascontiguousarray(W1).astype(BF),
        "w2": np.ascontiguousarray(W2).astype(BF),
        "bq": np.ascontiguousarray(bq, np.float32),
        "bk": np.ascontiguousarray(bk, np.float32),
        "b1": np.ascontiguousarray(b1, np.float32),
        "b2": np.ascontiguousarray(b2, np.float32),
        "g1": np.ascontiguousarray(gamma1, np.float32),
        "be1": np.ascontiguousarray(beta1, np.float32),
        "g2": np.ascontiguousarray(gamma2, np.float32),
        "be2": np.ascontiguousarray(beta2, np.float32),
    }
    if use_mask:
        mbias = np.where(mask, np.float32(0.0), np.float32(-1e12)).astype(np.float32)
    in_maps = []
    for c in range(N_CORES):
        b, j = divmod(c, 4)
        xb = x[b]
        m = dict(common)
        m["xT"] = np.ascontiguousarray(xb.T).astype(F8)
        m["xTq"] = np.ascontiguousarray(xb[j * TPC:(j + 1) * TPC].T).astype(F8)
        m["xres"] = np.ascontiguousarray(
            xb[j * TPC:(j + 1) * TPC] + res_bias[None, :])
        if use_mask:
            m["maskT"] = np.ascontiguousarray(mbias.T[:, j * TPC:(j + 1) * TPC])
        in_maps.append(m)
    return use_mask, in_maps


def assemble_output(results) -> np.ndarray:
    out = np.empty((2, S, D), np.float32)
    for c in range(N_CORES):
        b, j = divmod(c, 4)
        out[b, j * TPC:(j + 1) * TPC] = results[c]["out"]
    return out


def kernel(**inputs) -> np.ndarray:
    use_mask, in_maps = make_in_maps(**inputs)
    nc = _get_program(use_mask)
    res = run_bass_kernel_spmd(nc, in_maps, list(range(N_CORES)))
    return assemble_output(res.results)
